# revision 9
# baseline (speedup 1.0000x reference)
"""Trainium2 Bass kernel for nn_AttentionOut (causal MHA + output projection).

Problem: B=2, S=2048, D=1024, H=16 heads, Dh=64, fp32.
  out = softmax(causal(q@k^T/8)) @ v, projected by W_O, plus b_O.
  Returns (residual, out) like the reference.

Sharding (8 cores): batch x head-group tensor parallel. Core c handles
batch b=c//4 and heads [4g, 4g+4) with g=c%4. Each core computes its
4 heads' attention for all queries plus the partial output projection
(2048, 1024) in fp16; the host sums the 4 partials per batch in fp32
(the "all-reduce") and adds b_O.

Per-core device algorithm (fp16 operands, fp32 PSUM accumulation):
  - scores TRANSPOSED: sT[k, q] = sum_d kT[d,k] qT[d,q]; head pairs
    packed on the 128 partitions via row-tiled matmuls.
  - exp on ScalarE (scale=1/8 fused) into fp16 SBUF; causal mask on
    diagonal 128x128 blocks via 0/1 triangular mask multiply (DVE).
  - AV: PE throughput here is bound by output columns, so the softmax
    denominator rides for free where it can: heads h0/h1 use M=65
    matmuls (v plus a ones column -> z rows 0..63, den at row 64) into
    per-head PSUM tiles A/B; heads h2/h3 pack M=64 into one tile C
    (col positions 0/64) with explicit M=32 ones-matmul denominators
    accumulated into the spare rows 96..127 of A/B.
  - normalize: reciprocal_approx_fast (fp32) on the den rows, K=1
    ones-matmul broadcast (f32r) across partitions, DVE multiply to
    fp16 zn tiles stacked per projection pair [h0;h3], [h2;h1] (h1
    hops partitions via a small SBUF->SBUF DMA).
  - projection: out[q, m] = sum_pairs znP^T @ W_O (W_O rows reordered
    to match the zn pairing), interleaved into the next chunk's score
    loop; results DMA'd to DRAM as fp16.
"""

import sys
import numpy as np

sys.path.insert(0, "/opt/trn_rl_repo")

B, S, D = 2, 2048, 1024
NH, DH = 16, 64
P = 128
NCORES = 8
HPC = 4            # heads per core
NPAIR = 2          # head pairs per core
QCH = 512          # query chunk (columns of transposed scores)
NCHUNK = S // QCH  # 4
NJ = S // P        # 16 key blocks

_COMPILED = None


def _build():
    import concourse.bacc as bacc
    import concourse.mybir as mybir
    import concourse.tile as tile

    F32 = mybir.dt.float32
    F16 = mybir.dt.float16
    EXP = mybir.ActivationFunctionType.Exp

    nc = bacc.Bacc("TRN2", target_bir_lowering=False, debug=False,
                   num_devices=NCORES)

    qT_d = nc.dram_tensor("qT", [2 * P, S], F16, kind="ExternalInput")
    kT_d = nc.dram_tensor("kT", [2 * P, S], F16, kind="ExternalInput")
    v_d = nc.dram_tensor("v", [S, HPC, DH], F16, kind="ExternalInput")
    wo_d = nc.dram_tensor("wo", [2 * P, D], F16, kind="ExternalInput")
    tri_d = nc.dram_tensor("tri", [P, P], F16, kind="ExternalInput")
    out_d = nc.dram_tensor("outp", [S, D], F16, kind="ExternalOutput")

    with tile.TileContext(nc) as tc:
        with (
            tc.tile_pool(name="const", bufs=1) as cpool,
            tc.tile_pool(name="work", bufs=6) as wpool,
            tc.tile_pool(name="zn", bufs=4) as znpool,
            tc.tile_pool(name="nrm", bufs=2) as npool,
            tc.tile_pool(name="ost", bufs=3) as opool,
            tc.tile_pool(name="psc", bufs=2, space="PSUM") as psc,
            tc.tile_pool(name="pz", bufs=3, space="PSUM") as pz,
            tc.tile_pool(name="pd", bufs=1, space="PSUM") as pd,
        ):
            kT_sb = cpool.tile([P, NPAIR, S], F16, tag="kT")
            qT_sb = cpool.tile([P, NPAIR, S], F16, tag="qT")
            v_sb = cpool.tile([P, NJ, HPC, DH], F16, tag="v")
            wo_sb = cpool.tile([P, NPAIR, D], F16, tag="wo")
            tri_sb = cpool.tile([P, P], F16, tag="tri")
            ones_sb = cpool.tile([P, 32], F16, tag="ones")
            onesbc_sb = cpool.tile([P, 64], F16, tag="onesbc")

            # chunk-0 data first so compute starts ~2.5us in; the rest
            # streams behind chunks 0-1.
            v_re = v_d.rearrange("(j p) h e -> p j h e", p=P)
            nc.sync.dma_start(kT_sb[:, 0, 0:QCH], kT_d[0:P, 0:QCH])
            nc.sync.dma_start(qT_sb[:, 0, 0:QCH], qT_d[0:P, 0:QCH])
            nc.sync.dma_start(kT_sb[:, 1, 0:QCH], kT_d[P:2 * P, 0:QCH])
            nc.sync.dma_start(qT_sb[:, 1, 0:QCH], qT_d[P:2 * P, 0:QCH])
            nc.sync.dma_start(v_sb[:, 0:4], v_re[:, 0:4])
            nc.sync.dma_start(tri_sb[:], tri_d[:])
            nc.vector.memset(ones_sb[:], 1.0)
            nc.vector.memset(onesbc_sb[:], 1.0)
            nc.sync.dma_start(kT_sb[:, 0, QCH:S], kT_d[0:P, QCH:S])
            nc.sync.dma_start(qT_sb[:, 0, QCH:S], qT_d[0:P, QCH:S])
            nc.sync.dma_start(kT_sb[:, 1, QCH:S], kT_d[P:2 * P, QCH:S])
            nc.sync.dma_start(qT_sb[:, 1, QCH:S], qT_d[P:2 * P, QCH:S])
            nc.sync.dma_start(v_sb[:, 4:NJ], v_re[:, 4:NJ])
            nc.sync.dma_start(wo_sb[:], wo_d.rearrange("(c p) m -> p c m", p=P))

            # deferred projection work, interleaved into later chunks.
            proj_queue = []

            def emit_proj_one(reserve=2):
                if len(proj_queue) > reserve:
                    proj_queue.pop(0)()

            def make_proj(c, znP):
                def emit(qs, mc):
                    po = pd.tile([P, QCH], F32, tag="pd", name="po")
                    for pair in range(NPAIR):
                        nc.tensor.matmul(
                            po[:],
                            znP[pair][:, qs * P:(qs + 1) * P],
                            wo_sb[:, pair, mc * QCH:(mc + 1) * QCH],
                            start=(pair == 0), stop=(pair == 1),
                        )
                    ot = opool.tile([P, QCH], F16, tag="ot", name="ot")
                    with nc.allow_low_precision(
                            reason="fp16 partial output within budget"):
                        nc.vector.tensor_copy(ot[:], po[:])
                    nc.sync.dma_start(
                        out_d[c * QCH + qs * P: c * QCH + (qs + 1) * P,
                              mc * QCH:(mc + 1) * QCH],
                        ot[:])
                return [lambda qs=qs, mc=mc: emit(qs, mc)
                        for qs in range(QCH // P) for mc in range(D // QCH)]

            # chunk-boundary normalize, deferred into the next chunk's
            # pipeline so PE keeps streaming while the recips run.
            pending_norm = [None]

            def make_norm(c, zC1, zC2, zD):
                # dens: head h at zD row 32*h
                def run():
                    # zD is fully written (each M=32 den matmul emits 32
                    # identical rows), so full-tile ops are safe. The
                    # approx reciprocal must read SBUF: its custom-DVE op
                    # returns wrong data from PSUM on hardware.
                    dsb = npool.tile([P, QCH], F32, tag="dsb",
                                     name="dsb")
                    recf = npool.tile([P, QCH], F32, tag="recf",
                                      name="recf")
                    rech = npool.tile([P, QCH], F16, tag="rech",
                                      name="rech")
                    nc.vector.tensor_copy(dsb[:], zD[:])
                    nc.vector.reciprocal_approx_fast(
                        out=recf[:], in_=dsb[:])
                    with nc.allow_low_precision(
                            reason="fp16 softmax reciprocal in budget"):
                        nc.vector.tensor_copy(rech[:], recf[:])

                    znP = [znpool.tile([P, QCH], F16, tag="znP",
                                       name=f"znP_c{c}p{i}")
                           for i in range(NPAIR)]
                    lp = dict(reason="fp16 softmax normalize in budget")

                    for pair, zt in ((0, zC1), (1, zC2)):
                        bc = pd.tile([P, QCH], F32, tag="pd",
                                     name=f"bc{pair}")
                        for par in range(2):
                            h = 2 * pair + par
                            nc.tensor.matmul(
                                bc[64 * par:64 * par + 64, :],
                                onesbc_sb[32 * h:32 * h + 1, 0:64],
                                rech[32 * h:32 * h + 1, :],
                                start=True, stop=True,
                                tile_position=(32 * h, 64 * par),
                                skip_group_check=True)
                        bcs = npool.tile([P, QCH], F16, tag=f"bcs{pair}",
                                         name=f"bcs{pair}")
                        with nc.allow_low_precision(**lp):
                            nc.vector.tensor_copy(bcs[:], bc[:])
                            nc.vector.tensor_tensor(
                                znP[pair][0:64, :], zt[0:64, :],
                                bcs[0:64, :], mybir.AluOpType.mult)
                            nc.vector.tensor_tensor(
                                znP[pair][64:128, :], zt[64:128, :],
                                bcs[64:128, :], mybir.AluOpType.mult)

                    proj_queue.extend(make_proj(c, znP))
                return run

            for c in range(NCHUNK):
                jmax = 4 * (c + 1)
                zABC = [None, None, None]

                # merged pair loop: both pairs' scores/exp per j, then
                # (pipelined one j behind) the AV/den phase, ordered so
                # each adjacent matmul pair occupies disjoint PE column
                # bands and runs concurrently:
                #   [AVh0 | den_h3], [AVh1 | den_h2], [AVh2 | AVh3]
                pending = None

                def emit_avden(pj, pqoff, pqlen, pexpTs):
                    zC1, zC2, zD = zABC
                    e0, e1 = pexpTs
                    st = dict(start=(pj == 0), stop=(pj == jmax - 1),
                              skip_group_check=True)
                    # group: h0 | h1 (disjoint col bands of zC1)
                    nc.tensor.matmul(
                        zC1[0:64, pqoff:QCH],
                        v_sb[:, pj, 0, :], e0[:, 0, :pqlen],
                        tile_position=(0, 0), **st)
                    nc.tensor.matmul(
                        zC1[64:128, pqoff:QCH],
                        v_sb[:, pj, 1, :], e0[:, 1, :pqlen],
                        tile_position=(0, 64), **st)
                    # group: h2 | h3 (zC2)
                    nc.tensor.matmul(
                        zC2[0:64, pqoff:QCH],
                        v_sb[:, pj, 2, :], e1[:, 0, :pqlen],
                        tile_position=(0, 0), **st)
                    nc.tensor.matmul(
                        zC2[64:128, pqoff:QCH],
                        v_sb[:, pj, 3, :], e1[:, 1, :pqlen],
                        tile_position=(0, 64), **st)
                    # group: all 4 dens, 4-way col-band concurrency
                    for h, e, par in ((0, e0, 0), (1, e0, 1),
                                      (2, e1, 0), (3, e1, 1)):
                        nc.tensor.matmul(
                            zD[32 * h:32 * h + 32, pqoff:QCH],
                            ones_sb[:, 0:32], e[:, par, :pqlen],
                            tile_position=(0, 32 * h), **st)

                for j in range(jmax):
                    qoff = max(0, P * j - QCH * c)
                    qlen = QCH - qoff
                    diag = P * j >= QCH * c
                    q0 = QCH * c + qoff

                    expTs = []
                    for pair in range(NPAIR):
                        sc = psc.tile([P, 2, QCH], F32, tag="sc",
                                      name="sc")
                        for par in range(2):
                            nc.tensor.matmul(
                                sc[:, par, :qlen],
                                kT_sb[64 * par:64 * par + 64, pair,
                                      P * j:P * (j + 1)],
                                qT_sb[64 * par:64 * par + 64, pair,
                                      q0:q0 + qlen],
                                start=True, stop=True,
                                tile_position=(64 * par, 0),
                            )
                        expT = wpool.tile([P, 2, QCH], F16, tag="expT",
                                          name="expT")
                        nc.scalar.activation(
                            expT[:, :, :qlen], sc[:, :, :qlen], EXP,
                            scale=0.125)
                        if diag:
                            nc.vector.tensor_tensor(
                                expT[:, :, 0:P], expT[:, :, 0:P],
                                tri_sb[:, None, :].to_broadcast((P, 2, P)),
                                mybir.AluOpType.mult)
                        expTs.append(expT)
                    if j == 0:
                        # previous chunk's normalize runs here, hidden
                        # behind this chunk's first scores + reserved
                        # projection matmuls
                        if pending_norm[0] is not None:
                            emit_proj_one(reserve=0)
                            emit_proj_one(reserve=0)
                            pending_norm[0]()
                            pending_norm[0] = None
                        zABC[0] = pz.tile([P, QCH], F32, tag="z",
                                          name=f"zC1_c{c}")
                        zABC[1] = pz.tile([P, QCH], F32, tag="z",
                                          name=f"zC2_c{c}")
                        zABC[2] = pz.tile([P, QCH], F32, tag="z",
                                          name=f"zD_c{c}")
                    if pending is not None:
                        emit_avden(*pending)
                        emit_proj_one()
                    pending = (j, qoff, qlen, expTs)
                emit_avden(*pending)
                emit_proj_one()

                pending_norm[0] = make_norm(c, *zABC)

            pending_norm[0]()
            while proj_queue:
                proj_queue.pop(0)()

    nc.compile()
    return nc


def _prep_inputs(c, q, k, v, W_O):
    b, g = c // 4, c % 4
    hs = slice(g * HPC * DH, (g + 1) * HPC * DH)
    qT = np.ascontiguousarray(q[b][:, hs].T.astype(np.float16))
    kT = np.ascontiguousarray(k[b][:, hs].T.astype(np.float16))
    vh = np.ascontiguousarray(
        v[b][:, hs].reshape(S, HPC, DH).astype(np.float16))
    wo = np.ascontiguousarray(
        W_O[g * HPC:(g + 1) * HPC].reshape(HPC * DH, D).astype(np.float16))
    tri = np.triu(np.ones((P, P), dtype=np.float16))
    return {"qT": qT, "kT": kT, "v": vh, "wo": wo, "tri": tri}


def _get_compiled():
    global _COMPILED
    if _COMPILED is None:
        _COMPILED = _build()
    return _COMPILED


def kernel(residual, q, k, v, W_O, b_O, _trace=False, _trace_cores=None):
    from concourse.bass_utils import run_bass_kernel_spmd

    residual = np.asarray(residual, dtype=np.float32)
    q = np.asarray(q, dtype=np.float32)
    k = np.asarray(k, dtype=np.float32)
    v = np.asarray(v, dtype=np.float32)
    W_O = np.asarray(W_O, dtype=np.float32)
    b_O = np.asarray(b_O, dtype=np.float32)

    nc = _get_compiled()
    core_ids = list(range(NCORES))
    in_maps = [_prep_inputs(c, q, k, v, W_O) for c in core_ids]
    kw = {}
    if _trace:
        kw = dict(trace=True,
                  trace_cores=_trace_cores or core_ids)
    res = run_bass_kernel_spmd(nc, in_maps, core_ids, **kw)

    out = np.zeros((B, S, D), dtype=np.float32)
    for c in core_ids:
        out[c // 4] += res.results[c]["outp"].astype(np.float32)
    out += b_O
    if _trace:
        kernel.last_result = res
    return (residual, out)
